# revision 7
# baseline (speedup 1.0000x reference)
"""Trainium2 Bass kernel for nn_DualImageAttnProcessor.

Math (per batch element b, sharded one batch element per NeuronCore):
    q = hidden @ Wq                                   [4096, 1280]
    For each branch (text/id/hair): k_b, v_b projections, then
    out_b = softmax(q k_b^T / 8) v_b per head.
    result = (out_text + out_id + out_hair) @ Wo + 3*b_o
(Wo/b_o are shared across branches with unit scales, so the three branch
outputs are summed BEFORE the output projection.)

Device-side layout is fully transposed ([channels, seq] with channels on
partitions) so every matmul has its contraction dim on partitions:
    QT  = Wq^T X^T        via matmul(lhsT=Wq, rhs=XT)
    KTs = per-branch K^T packed along Sk -> [1280, 109]
    Vs  = per-branch V packed along partitions -> [109, 1280]
    scoresT_h = KT_h^T-slice.T @ QT_h = (K Q^T)    [109, 512-chunk]
    softmax per branch along the partition dim:
        exp on ACT; per-branch column sums via a 0/1-mask matmul on PE;
        reciprocal on DVE; broadcast recips back to [109, n] via a second
        tiny (K=3) matmul on PE; normalize with one DVE multiply.
    OT_h = Vs_h^T @ Pn  (sums all three branches in one K=109 matmul)
    YT  = Wo^T OT         -> host transposes back and adds 3*b_o.
"""

import numpy as np
import ml_dtypes
from contextlib import ExitStack

import concourse.bass as bass
import concourse.bacc as bacc
import concourse.mybir as mybir
import concourse.tile as tile
from concourse.bass_utils import run_bass_kernel_spmd
from concourse.masks import make_identity

# Problem shapes (hardcoded per contest contract)
B, SQ, STX, SI = 8, 4096, 77, 16
C, DC, H, DH = 1280, 2048, 20, 64
SCALE = 1.0 / 8.0
SK = STX + 2 * SI                     # 109 packed kv positions
OFFS = (0, STX, STX + SI)             # branch offsets in packed Sk
SKS = (STX, SI, SI)
P = 128
CHUNK = 512
NCH = SQ // CHUNK                     # 8
MT = C // P                           # 10 channel tiles
KTC = C // P                          # 10 contraction tiles over C
KTD = DC // P                         # 16 contraction tiles over DC
NG = H // 2                           # 10 head-pair groups per chunk

BF16 = mybir.dt.bfloat16
F32 = mybir.dt.float32
EXP = mybir.ActivationFunctionType.Exp
MULT = mybir.AluOpType.mult


def _build_nc():
    nc = bacc.Bacc(None, target_bir_lowering=False)

    xt = nc.dram_tensor("xt", [C, SQ], BF16, kind="ExternalInput")
    st = nc.dram_tensor("st", [DC, SK], BF16, kind="ExternalInput")
    wq = nc.dram_tensor("wq", [C, C], BF16, kind="ExternalInput")
    wo = nc.dram_tensor("wo", [C, C], BF16, kind="ExternalInput")
    wk3 = nc.dram_tensor("wk3", [3, DC, C], BF16, kind="ExternalInput")
    wv3 = nc.dram_tensor("wv3", [3, DC, C], BF16, kind="ExternalInput")
    mask3_d = nc.dram_tensor("mask3", [SK, 3], BF16, kind="ExternalInput")
    m3t_d = nc.dram_tensor("m3t", [35, SK], F32, kind="ExternalInput")
    yt = nc.dram_tensor("yt", [C, SQ], BF16, kind="ExternalOutput")

    xt_r = xt.rearrange("(t p) n -> p t n", p=P)
    st_r = st.rearrange("(t p) s -> p t s", p=P)
    wq_r = wq.rearrange("(t p) n -> p t n", p=P)
    wo_r = wo.rearrange("(t p) n -> p t n", p=P)
    yt_r = yt.rearrange("(t p) n -> p t n", p=P)

    with tile.TileContext(nc) as tc, ExitStack() as ctx:
        sb = ctx.enter_context(tc.tile_pool(name="sb", bufs=1))
        ps = ctx.enter_context(tc.tile_pool(name="ps", bufs=1, space="PSUM"))

        # ---- constants (host-provided; engine writes at partition offsets
        # 77/93 are illegal, DMA writes are fine) ----
        ident = sb.tile([P, P], BF16, name="ident", tag="ident")
        make_identity(nc, ident)
        # mask3[s, b] = 1 if kv position s belongs to branch b
        mask3 = sb.tile([SK, 3], BF16, name="mask3", tag="mask3")
        nc.sync.dma_start(mask3[:], mask3_d[:])
        # m3t: mask3^T replicated at partition bases 0 and 32
        m3t = sb.tile([35, SK], F32, name="m3t", tag="m3t")
        nc.sync.dma_start(m3t[:], m3t_d[:])

        # ---- resident tensors ----
        wq_sb = sb.tile([P, KTC, C], BF16, name="wq_sb", tag="wq")
        nc.sync.dma_start(wq_sb[:], wq_r)
        wo_sb = sb.tile([P, KTC, C], BF16, name="wo_sb", tag="wo")
        nc.sync.dma_start(wo_sb[:], wo_r)
        st_sb = sb.tile([P, KTD, SK], BF16, name="st_sb", tag="st")
        nc.sync.dma_start(st_sb[:], st_r)

        kt_sb = sb.tile([P, MT, SK], BF16, name="kt_sb", tag="kt")
        v_sb = sb.tile([SK, C], BF16, name="v_sb", tag="vst")

        # ---- K/V projections (packed along Sk / partitions) ----
        for b3 in range(3):
            o, sk = OFFS[b3], SKS[b3]
            for which, wdram in (("k", wk3), ("v", wv3)):
                wdram_r = wdram[b3].rearrange("(t p) n -> p t n", p=P)
                for nch in range(0, C, CHUNK):
                    ncols = min(CHUNK, C - nch)
                    wkv_sb = sb.tile([P, KTD, CHUNK], BF16, name="wkv_sb",
                                     tag="wkv", bufs=2)
                    nc.sync.dma_start(wkv_sb[:, :, :ncols],
                                      wdram_r[:, :, nch:nch + ncols])
                    kv_ps = ps.tile([P, CHUNK], F32, name="kv_ps",
                                    tag="proj", bufs=2)
                    for k in range(KTD):
                        nc.tensor.matmul(kv_ps[:sk, :ncols],
                                         st_sb[:, k, o:o + sk],
                                         wkv_sb[:, k, :ncols],
                                         start=(k == 0), stop=(k == KTD - 1))
                    if which == "v":
                        if o == 0:
                            nc.scalar.copy(v_sb[:sk, nch:nch + ncols],
                                           kv_ps[:sk, :ncols])
                        else:
                            # engine writes must start at partition 0/32/64/96;
                            # stage at base 0, DMA into the packed rows
                            if nch == 0:
                                vstg = sb.tile([SI, C], BF16, name="vstg",
                                               tag="vstg", bufs=2)
                            nc.scalar.copy(vstg[:, nch:nch + ncols],
                                           kv_ps[:sk, :ncols])
                            if nch + ncols == C:
                                nc.sync.dma_start(v_sb[o:o + sk, :], vstg[:])
                    else:
                        knat_sb = sb.tile([SK, CHUNK], BF16, name="knat_sb",
                                          tag="knat", bufs=2)
                        nc.scalar.copy(knat_sb[:sk, :ncols], kv_ps[:sk, :ncols])
                        for mm in range(ncols // P):
                            m = (nch + mm * P) // P
                            kt_ps = ps.tile([P, SK], BF16, name="kt_ps",
                                            tag="proj", bufs=2)
                            nc.tensor.transpose(kt_ps[:, :sk],
                                                knat_sb[:sk, mm * P:(mm + 1) * P],
                                                ident[:sk, :sk])
                            nc.vector.tensor_copy(kt_sb[:, m, o:o + sk],
                                                  kt_ps[:, :sk])

        # ---- main loop over 512-wide query chunks ----
        for c in range(NCH):
            cs = slice(c * CHUNK, (c + 1) * CHUNK)
            xt_sb = sb.tile([P, KTC, CHUNK], BF16, name="xt_sb",
                            tag="xt", bufs=2)
            nc.sync.dma_start(xt_sb[:], xt_r[:, :, cs])

            # QT chunk = Wq^T X^T
            qt_sb = sb.tile([P, MT, CHUNK], BF16, name="qt_sb",
                            tag="qt", bufs=2)
            for m in range(MT):
                qt_ps = ps.tile([P, CHUNK], F32, name="qt_ps",
                                tag="proj", bufs=2)
                for k in range(KTC):
                    nc.tensor.matmul(qt_ps[:], wq_sb[:, k, m * P:(m + 1) * P],
                                     xt_sb[:, k, :],
                                     start=(k == 0), stop=(k == KTC - 1))
                nc.scalar.copy(qt_sb[:, m, :], qt_ps[:])

            # attention, head pairs
            ot_sb = sb.tile([P, MT, CHUNK], BF16, name="ot_sb",
                            tag="ot", bufs=2)
            for g in range(NG):
                # scoresT for the two heads of this group
                s_ps = ps.tile([SK, 2, CHUNK], F32, name="s_ps",
                               tag="sgrp", bufs=2)
                for j in range(2):
                    h = 2 * g + j
                    p0 = DH * (h % 2)
                    nc.tensor.matmul(s_ps[:, j, :],
                                     kt_sb[p0:p0 + DH, h // 2, :],
                                     qt_sb[p0:p0 + DH, h // 2, :])
                e_sb = sb.tile([SK, 2, CHUNK], BF16, name="e_sb",
                               tag="esb", bufs=3)
                nc.scalar.activation(e_sb[:], s_ps[:], EXP, scale=SCALE)
                # per-branch sums at partition bases 0 and 32
                sum_ps = ps.tile([P, CHUNK], F32, name="sum_ps",
                                 tag="spv", bufs=2)
                for j in range(2):
                    nc.tensor.matmul(sum_ps[32 * j:32 * j + 3, :],
                                     mask3[:], e_sb[:, j, :])
                rcp_sb = sb.tile([35, CHUNK], F32, name="rcp_sb",
                                 tag="rcp", bufs=2)
                nc.vector.reciprocal(rcp_sb[:], sum_ps[:35, :])
                # broadcast recips back to [SK, CHUNK] (reuses s_ps banks)
                for j in range(2):
                    nc.tensor.matmul(s_ps[:, j, :], m3t[32 * j:32 * j + 3, :],
                                     rcp_sb[32 * j:32 * j + 3, :])
                pn_sb = sb.tile([SK, 2, CHUNK], BF16, name="pn_sb",
                                tag="pn", bufs=3)
                nc.vector.tensor_tensor(pn_sb[:], e_sb[:], s_ps[:], MULT)
                # PV for both heads into one [128, CHUNK] bank (reuses sum_ps)
                for j in range(2):
                    h = 2 * g + j
                    nc.tensor.matmul(sum_ps[DH * j:DH * (j + 1), :],
                                     v_sb[:, h * DH:(h + 1) * DH],
                                     pn_sb[:, j, :])
                nc.scalar.copy(ot_sb[:, g, :], sum_ps[:])

            # output projection YT = Wo^T OT
            y_sb = sb.tile([P, MT, CHUNK], BF16, name="y_sb", tag="ysb")
            for m in range(MT):
                y_ps = ps.tile([P, CHUNK], F32, name="y_ps",
                               tag="proj", bufs=2)
                for k in range(KTC):
                    nc.tensor.matmul(y_ps[:], wo_sb[:, k, m * P:(m + 1) * P],
                                     ot_sb[:, k, :],
                                     start=(k == 0), stop=(k == KTC - 1))
                nc.vector.tensor_copy(y_sb[:, m, :], y_ps[:])
            nc.sync.dma_start(yt_r[:, :, cs], y_sb[:])

    nc.compile()
    return nc


_NC_CACHE = None


def _get_nc():
    global _NC_CACHE
    if _NC_CACHE is None:
        _NC_CACHE = _build_nc()
    return _NC_CACHE


def _make_in_maps(hidden_states, text_states, id_states, hair_states,
                  Wq, Wk, Wv, Wo, Wk_id, Wv_id, Wk_hair, Wv_hair):
    bf = ml_dtypes.bfloat16
    f = lambda x: np.ascontiguousarray(np.asarray(x, dtype=np.float32)).astype(bf)
    wq_b = f(Wq)
    wo_b = f(Wo)
    wk3_b = np.stack([f(Wk), f(Wk_id), f(Wk_hair)])
    wv3_b = np.stack([f(Wv), f(Wv_id), f(Wv_hair)])
    hs = np.asarray(hidden_states, dtype=np.float32)
    ts = np.asarray(text_states, dtype=np.float32)
    ids = np.asarray(id_states, dtype=np.float32)
    hr = np.asarray(hair_states, dtype=np.float32)
    mask3 = np.zeros((SK, 3), np.float32)
    for b3 in range(3):
        mask3[OFFS[b3]:OFFS[b3] + SKS[b3], b3] = 1.0
    m3t = np.zeros((35, SK), np.float32)
    m3t[0:3, :] = mask3.T
    m3t[32:35, :] = mask3.T
    mask3_b = mask3.astype(bf)
    in_maps = []
    for b in range(B):
        stc = np.concatenate([ts[b], ids[b], hr[b]], axis=0).T  # [DC, SK]
        in_maps.append({
            "xt": np.ascontiguousarray(hs[b].T).astype(bf),
            "st": np.ascontiguousarray(stc).astype(bf),
            "wq": wq_b, "wo": wo_b, "wk3": wk3_b, "wv3": wv3_b,
            "mask3": mask3_b, "m3t": m3t,
        })
    return in_maps


def _run(in_maps, trace=False):
    nc = _get_nc()
    return run_bass_kernel_spmd(nc, in_maps, core_ids=list(range(B)),
                                trace=trace)


def kernel(hidden_states, text_states, id_states, hair_states,
           Wq, Wk, Wv, Wo, b_o, Wk_id, Wv_id, Wk_hair, Wv_hair):
    in_maps = _make_in_maps(hidden_states, text_states, id_states, hair_states,
                            Wq, Wk, Wv, Wo, Wk_id, Wv_id, Wk_hair, Wv_hair)
    res = _run(in_maps, trace=False)
    out = np.stack([r["yt"].astype(np.float32).T for r in res.results])
    out += 3.0 * np.asarray(b_o, dtype=np.float32)[None, None, :]
    return out
